# revision 5
# baseline (speedup 1.0000x reference)
"""Deformable Conv2d (B=4, Cin=128, Cout=256, H=W=64, K=3, s=1, p=1) on 8 trn2 cores.

Sharding: core = 2*b + h  (batch b, row-half h: rows h*32 .. h*32+31).
Per-core pipeline:
  - offset/mask 3x3 conv on PE (bf16, strided reads from one padded x copy),
    plus a rank-3 "affine tap" that folds the sampling-grid base coords and
    offset biases into the same PSUM accumulation
  - DVE/ACT scalar pipeline (bf16 back-half) -> bilinear corner coefs (bf16)
    + int16 gather idx (clipped to the canvas for memory safety)
  - dma_gather (per tap/row-half) from a host-built padded channels-last
    row-pair canvas in HBM: one 1KB element = 2x2 corner patch x 128 ch (bf16)
  - coef replication across partitions: mix of DVE shuffle / gpsimd
    partition_broadcast / stride-0 DMA broadcast (balances engines)
  - bf16 combine (coef x corners) on DVE
  - main matmul: 9 taps x 2 Cout tiles x 2 corner-pair streams (some units
    feed 4 streams, skipping the DVE pair-add), bf16, PSUM-accumulated ->
    bf16 out
"""
import numpy as np
import ml_dtypes
from contextlib import ExitStack

import concourse.bacc as bacc
import concourse.bass as bass
import concourse.mybir as mybir
import concourse.tile as tile
from concourse import library_config
from concourse.bass_utils import run_bass_kernel_spmd

B, CIN, COUT, H, W, K = 4, 128, 256, 64, 64, 3
KK = K * K
NCORES = 8
HALF = H // 2            # 32 rows per core
N = HALF * W             # 2048 output positions per core
CH = 512                 # matmul chunk size (PSUM bank limit, fp32)
NCHUNK = N // CH
PCH = 512                # scalar-pipeline chunk size
NPCH = N // PCH
PW = PCH // 16           # idx-wrap column block per chunk
PADC = 18                # canvas padding (covers reference clip of +-16 + tap + bilinear)
HC = 100                 # canvas row-pairs  (y' = y + PADC, y in [-18, 81])
WC = 104                 # canvas cols (x' = x + PADC)
ES = 512                 # gather elem size in bf16 elements (1KB): 2x2 patch x 128ch
F32 = mybir.dt.float32
BF16 = mybir.dt.bfloat16
I16 = mybir.dt.int16
BF = ml_dtypes.bfloat16

_cache = {}

# f32->int16 convert on HW DVE rounds-to-nearest-even -> floor(t) = int(t-0.5).
# CoreSim models truncation -> floor(t) = int(t). Sim checks set this to 0.0.
FLOOR_DELTA = -0.5
# canvas-index clip (memory safety; reference's +-16 offset clip guarantees
# in-range indices for any plausible input, so this never binds)
CLIP_HI = 98.0
# per-(half, tap) replication engine: V=DVE shuffle, P=Pool broadcast, D=DMA
REPL = "PVDPVDPVP" "PVDPVPDVP"  # u = hf*9 + kk  (V=6, D=4, P=8)
# units where PE absorbs all 4 corner streams (skips the DVE pair-add)
PRT4 = "001010010" "010010010"
NWARM = 7                # PE pstate-ramp matmuls (spans the 3us ramp window)
NFILL = 0                # PE pstate-keepalive fillers per tap
NPRE = 0                 # PE fillers bridging the conv -> first-combine gap


def _build_program(debug=False):
    nc = bacc.Bacc("TRN2", target_bir_lowering=False, debug=False,
                   enable_asserts=False, num_devices=NCORES)
    if debug:
        dbg_t_d = nc.dram_tensor("dbg_t", [128, N], F32, kind="ExternalOutput")
        dbg_idx_d = nc.dram_tensor("dbg_idx", [128, N], I16, kind="ExternalOutput")
        dbg_ct_d = nc.dram_tensor("dbg_ct", [128, 4 * N], BF16,
                                  kind="ExternalOutput")
        dbg_s_d = nc.dram_tensor("dbg_s", [KK, 128, N], BF16,
                                 kind="ExternalOutput")
    xpad_d = nc.dram_tensor("xpad", [128, 34 * 66], BF16, kind="ExternalInput")
    canvas_d = nc.dram_tensor("canvas", [HC * WC + 1, ES // 2], BF16,
                              kind="ExternalInput")
    womT_d = nc.dram_tensor("womT", [128, (KK + 1) * 128], BF16,
                            kind="ExternalInput")
    wmnT_d = nc.dram_tensor("wmnT", [128, KK * 2 * 128], BF16, kind="ExternalInput")
    biasmsk_d = nc.dram_tensor("biasmsk", [128, 1], F32, kind="ExternalInput")
    arhs_d = nc.dram_tensor("arhs", [4, N], BF16, kind="ExternalInput")
    out_d = nc.dram_tensor("out", [2, 128, N], BF16, kind="ExternalOutput")

    with tile.TileContext(nc) as tc, ExitStack() as ctx:
        cpool = ctx.enter_context(tc.tile_pool(name="const", bufs=1))
        ppool = ctx.enter_context(tc.tile_pool(name="pipe", bufs=1))
        gpool = ctx.enter_context(tc.tile_pool(name="gath", bufs=8))
        rpool = ctx.enter_context(tc.tile_pool(name="crep", bufs=6))
        opool = ctx.enter_context(tc.tile_pool(name="outp", bufs=2))
        dpool = ctx.enter_context(tc.tile_pool(name="dram", bufs=1, space="DRAM"))
        pom_pool = ctx.enter_context(tc.tile_pool(name="psum", bufs=8, space="PSUM"))

        nc.gpsimd.load_library(library_config.mlp)

        # ---- load constants/inputs (conv deps first) ----
        xpr = xpad_d[:].rearrange("p (a b) -> p a b", a=34)
        xpad = cpool.tile([128, 34, 66], BF16, tag="xpad")
        nc.sync.dma_start(xpad[:, 0:18, :], xpr[:, 0:18, :])
        womT = cpool.tile([128, KK + 1, 128], BF16, tag="womT")
        nc.sync.dma_start(womT[:], womT_d[:].rearrange("p (t m) -> p t m", t=KK + 1))
        arhs = cpool.tile([4, N], BF16, tag="arhs")
        nc.sync.dma_start(arhs[:], arhs_d[:])
        biasmsk = cpool.tile([128, 1], F32, tag="biasmsk")
        nc.sync.dma_start(biasmsk[:], biasmsk_d[:])
        nc.sync.dma_start(xpad[:, 18:34, :], xpr[:, 18:34, :])
        wmnT = cpool.tile([128, KK * 2, 128], BF16, tag="wmnT")
        nc.sync.dma_start(wmnT[:], wmnT_d[:].rearrange("p (t m) -> p t m", t=KK * 2))

        warm = cpool.tile([128, CH], BF16, tag="warm")
        nc.gpsimd.memset(warm[:], 0.0)
        wps = pom_pool.tile([128, CH], F32, tag="ps")
        for _w in range(NWARM):
            nc.tensor.matmul(wps[:], warm[:, 0:128], warm[:],
                             start=(_w == 0), stop=(_w == NWARM - 1))
        # prefetch the sigmoid activation table under the warmup
        sigpre = cpool.tile([128, 1], F32, tag="sigpre")
        nc.scalar.activation(sigpre[:], warm[:, 0:1],
                             mybir.ActivationFunctionType.Sigmoid)

        idx_t = cpool.tile([128, N], I16, tag="idx")
        ct = cpool.tile([128, 4, N], BF16, tag="coef")
        wrap = cpool.tile([128, 2, KK, 64], I16, tag="wrap")
        wrapd = dpool.tile([16, 2 * KK * 64], I16, tag="wrapd")
        ctd = dpool.tile([KK, 4 * N], BF16, tag="ctd")

        maskx = [9 + i if i <= 22 else 31 for i in range(32)]
        maskm = [18 + i if i <= 13 else 31 for i in range(32)]
        AL = mybir.AluOpType
        AF = mybir.ActivationFunctionType

        # ---- offset/mask conv + scalar pipeline ----
        # Two passes per chunk-pair: pass A computes the gather indices for
        # both chunks (so the wrap DMA + first gather launch early); pass B
        # computes the bilinear coefs.
        def pipe_a(c):
            sl = slice(c * PCH, (c + 1) * PCH)
            pom = pom_pool.tile([128, CH], F32, tag="ps")
            r0 = 8 * c
            for t in range(KK):
                ky, kx = t // 3, t % 3
                rhs = xpad[:, r0 + ky: r0 + ky + 8, kx:kx + 64]
                nc.tensor.matmul(pom[:], womT[:, t, :], rhs,
                                 start=(t == 0), stop=False)
            nc.tensor.matmul(pom[:], womT[0:4, KK, :], arhs[:, sl],
                             start=False, stop=True)
            # t = conv + bias + base (f32), into SBUF via ACT
            tsb = ppool.tile([128, PCH], F32, tag=f"tsb{c % 2}")
            nc.scalar.copy(tsb[:], pom[:])
            if debug:
                nc.sync.dma_start(dbg_t_d[:, sl], tsb[:])
            # mask m/2 = sigmoid(t' + mod_b) (bf16); 2x folded into wmnT
            mhat = ppool.tile([128, PCH], BF16, tag=f"mhat{c % 2}")
            nc.scalar.activation(mhat[:], pom[:], AF.Sigmoid, bias=biasmsk[:],
                                 scale=1.0)
            # floor(t) as int16, clipped to canvas bounds
            f0a = ppool.tile([128, PCH], I16, tag=f"f0a{c % 2}")
            nc.vector.tensor_scalar(f0a[:], tsb[:], FLOOR_DELTA, 0.0,
                                    AL.add, AL.max)
            f0i = ppool.tile([128, PCH], I16, tag=f"f0i{c % 2}")
            nc.vector.tensor_scalar(f0i[:], f0a[:], CLIP_HI, None, AL.min)
            f0 = ppool.tile([128, PCH], F32, tag=f"f0{c % 2}")
            nc.scalar.copy(f0[:], f0i[:])
            # x-floor aligned to y rows (int16, shuffled as f32 bit pairs)
            f0xal = ppool.tile([128, PCH], I16, tag=f"f0xal{c % 2}")
            nc.vector.stream_shuffle(f0xal[:].bitcast(F32), f0i[:].bitcast(F32),
                                     maskx)
            # idx written at transposed positions tau(q) = 128*(q%16) + q//16
            iap = idx_t[:]
            idx_dst = bass.AP(iap.tensor, iap.offset + PW * c,
                              [iap.ap[0], [1, PW], [128, 16]])
            nc.vector.scalar_tensor_tensor(idx_dst, f0i[:], float(WC), f0xal[:],
                                           AL.mult, AL.add)
            return tsb, mhat, f0

        def pipe_b(c, tsb, mhat, f0):
            sl = slice(c * PCH, (c + 1) * PCH)
            # fractional part, straight to bf16
            fr = ppool.tile([128, PCH], BF16, tag=f"fr{c % 2}")
            nc.vector.tensor_tensor(fr[:], tsb[:], f0[:], AL.subtract)
            # mask + x-frac aligned to y rows (bf16 shuffles via f32 bitcast)
            mhal = ppool.tile([128, PCH], BF16, tag=f"mhal{c % 2}")
            nc.vector.stream_shuffle(mhal[:].bitcast(F32), mhat[:].bitcast(F32),
                                     maskm)
            fxal = ppool.tile([128, PCH], BF16, tag=f"fxal{c % 2}")
            nc.vector.stream_shuffle(fxal[:].bitcast(F32), fr[:].bitcast(F32),
                                     maskx)
            omfx = ppool.tile([128, PCH], BF16, tag=f"omfx{c % 2}")
            nc.scalar.activation(omfx[:], fxal[:], AF.Copy, bias=1.0, scale=-1.0)
            my1 = ppool.tile([128, PCH], BF16, tag=f"my1{c % 2}")
            nc.vector.tensor_tensor(my1[:], mhal[:], fr[:], AL.mult)
            my0 = ppool.tile([128, PCH], BF16, tag=f"my0{c % 2}")
            nc.vector.tensor_tensor(my0[:], mhal[:], my1[:], AL.subtract)
            nc.vector.tensor_tensor(ct[:, 0, sl], my0[:], omfx[:], AL.mult)
            nc.vector.tensor_tensor(ct[:, 1, sl], my1[:], omfx[:], AL.mult)
            nc.vector.tensor_tensor(ct[:, 2, sl], my0[:], fxal[:], AL.mult)
            nc.vector.tensor_tensor(ct[:, 3, sl], my1[:], fxal[:], AL.mult)
            # stage this chunk's coefs to DRAM (for D-mode replication)
            dstc = bass.AP(ctd[:].tensor, ctd[:].offset + c * PCH,
                           [[4 * N, KK], [N, 4], [1, PCH]])
            nc.sync.dma_start(dstc, ct[0:KK, :, sl])

        def wrap_stage(hfc):
            # wrap idx into dma_gather layout via a DRAM roundtrip
            # (1 DMA out, 1 stride-0 replicating DMA back)
            iap = idx_t[0:KK, :]
            src = bass.AP(iap.tensor, iap.offset + 64 * hfc,
                          [iap.ap[0], [128, 16], [1, 64]])
            dap = wrapd[:]
            dst = bass.AP(dap.tensor, dap.offset + hfc * KK * 64,
                          [[64, KK], [2 * KK * 64, 16], [1, 64]])
            nc.sync.dma_start(dst, src)
            wsrc = bass.AP(dap.tensor, dap.offset + hfc * KK * 64,
                           [[0, 8], [2 * KK * 64, 16], [1, KK * 64]])
            nc.sync.dma_start(wrap[:, hfc, :, :], wsrc)

        for cp in range(NPCH // 2):
            st0 = pipe_a(2 * cp)
            st1 = pipe_a(2 * cp + 1)
            wrap_stage(cp)
            pipe_b(2 * cp, *st0)
            pipe_b(2 * cp + 1, *st1)

        if debug:
            nc.sync.dma_start(dbg_idx_d[:], idx_t[:])
            nc.sync.dma_start(dbg_ct_d[:], ct[:].rearrange("p a b -> p (a b)"))

        # ---- per (half, tap): gather + coef replication + combine + matmul ----
        # Gathers fetch 3 taps per call (amortizes SWDGE fixed overhead).
        # Main-matmul accumulation is kk-outer: all 8 (m, chunk) PSUM banks
        # stay open across the kk loop so PE work rides along the gathers.
        N2 = N // 2
        cap = canvas_d[:]
        cview = bass.AP(cap.tensor, cap.offset, [[ES // 2, HC * WC], [1, ES]])
        pstiles = []
        for _i in range(8):
            pst = pom_pool.tile([128, CH], F32, tag="ps")
            pstiles.append(pst)
        # bridge the conv -> first-combine PE gap (keeps the pstate ramp hot);
        # these pre-start bank 0, so its first real matmul uses start=False
        for _f in range(NPRE):
            nc.tensor.matmul(pstiles[0][:], warm[:, 0:128], warm[:],
                             start=(_f == 0), stop=False)
        for hf in range(2):
            hsl = slice(hf * N2, (hf + 1) * N2)
            for kk in range(KK):
                G = gpool.tile([128, 4, N2], BF16, tag="G")
                nc.gpsimd.dma_gather(
                    G[:], cview, wrap[:, hf, kk, :],
                    N2, N2, ES, elem_step=ES // 2, transpose=True,
                    single_packet=False)
                crep = rpool.tile([128, 4, N2], BF16, tag="crep")
                mode = REPL[hf * KK + kk]
                if mode == "V":
                    nc.vector.stream_shuffle(crep[:].bitcast(F32),
                                             ct[:, :, hsl].bitcast(F32),
                                             [kk] * 32)
                elif mode == "P":
                    p0 = opool.tile([1, 4 * N2], BF16, tag="p0stage")
                    nc.sync.dma_start(p0[:], ct[kk: kk + 1, :, hsl])
                    nc.gpsimd.partition_broadcast(
                        crep[:].rearrange("p a b -> p (a b)"), p0[:])
                else:
                    dap = ctd[kk: kk + 1, :]
                    src = bass.AP(dap.tensor, dap.offset + hf * N2,
                                  [[0, 128], [N, 4], [1, N2]])
                    nc.sync.dma_start(crep[:], src)
                # the very last unit combines per 512-chunk for a shorter tail
                ccsplit = (hf == 1 and kk == KK - 1)
                nprt = 4 if PRT4[hf * KK + kk] == "1" else 2
                for cs in ([0, 1] if ccsplit else [None]):
                    if cs is None:
                        Gv, cr, ccr = G[:], crep[:], range(N2 // CH)
                    else:
                        Gv = G[:, :, cs * CH:(cs + 1) * CH]
                        cr = crep[:, :, cs * CH:(cs + 1) * CH]
                        ccr = [cs]
                    nc.vector.tensor_tensor(Gv, Gv, cr, AL.mult)
                    if nprt == 2:
                        nc.vector.tensor_tensor(Gv[:, 0:2, :], Gv[:, 0:2, :],
                                                Gv[:, 2:4, :], AL.add)
                    for m in range(2):
                        for cc in ccr:
                            for prt in range(nprt):
                                st = (kk == 0 and prt == 0
                                      and not (NPRE > 0 and m == 0 and hf == 0
                                               and cc == 0))
                                nc.tensor.matmul(
                                    pstiles[m * NCHUNK + hf * (N2 // CH) + cc][:],
                                    wmnT[:, kk * 2 + m, :],
                                    G[:, prt, cc * CH:(cc + 1) * CH],
                                    start=st,
                                    stop=(kk == KK - 1 and prt == nprt - 1))
                if debug:
                    s = opool.tile([128, N2], BF16, tag="sdbg")
                    nc.vector.tensor_tensor(s[:], G[:, 0, :], G[:, 1, :],
                                            AL.add)
                    nc.sync.dma_start(dbg_s_d[kk, :, hsl], s[:])
                # pstate keepalive: zero-adds keep the PE ramp hot while the
                # next gather lands (omit at kk=8 where banks are closing)
                if kk < KK - 1:
                    for _f in range(NFILL):
                        nc.tensor.matmul(
                            pstiles[hf * (N2 // CH)][:], warm[:, 0:128],
                            warm[:], start=False, stop=False)

            if hf == 0:
                for m in range(2):
                    for c in range(2):
                        ob = opool.tile([128, CH], BF16, tag="ob")
                        nc.scalar.copy(ob[:], pstiles[m * NCHUNK + c][:])
                        nc.sync.dma_start(out_d[m, :, c * CH:(c + 1) * CH],
                                          ob[:])
        # tail: drain closing banks via one staged tile + a single DMA,
        # copies split across Act and DVE
        ob4 = opool.tile([128, 2, 2, CH], BF16, tag="ob4")
        for c in range(2, NCHUNK):
            for m in range(2):
                if m == 0:
                    nc.scalar.copy(ob4[:, m, c - 2, :],
                                   pstiles[m * NCHUNK + c][:])
                else:
                    nc.vector.tensor_copy(ob4[:, m, c - 2, :],
                                          pstiles[m * NCHUNK + c][:])
        oap = out_d[:]
        for c2 in range(2):
            odst = bass.AP(oap.tensor, oap.offset + (2 + c2) * CH,
                           [[N, 128], [128 * N, 2], [1, CH]])
            nc.sync.dma_start(odst, ob4[:, :, c2, :])

    nc.compile()
    return nc


def _prep_core_inputs(x, offset_w, offset_b, mod_w, mod_b, weight, b, h):
    """Host-side layout prep for core (b, h). Pure reshaping/padding/casting."""
    f32 = np.float32
    # xpad: rows h*32-1..h*32+32 zero-padded, 66 wide (cols -1..64), bf16
    xpad = np.zeros((128, 34, 66), f32)
    r0 = h * HALF - 1
    lo = max(0, -r0)
    hi = min(34, H - r0)
    xpad[:, lo:hi, 1:65] = x[b, :, r0 + lo:r0 + hi, :]
    xpad = xpad.astype(BF)
    # canvas: padded channels-last row-pair canvas (per batch), bf16
    xcl = np.ascontiguousarray(x[b].transpose(1, 2, 0)).astype(BF)    # [64,64,128]
    padded = np.zeros((101, WC, 128), BF)
    padded[PADC:PADC + H, PADC:PADC + W, :] = xcl
    canvas = np.concatenate([padded[:-1], padded[1:]], axis=2)        # [100,104,256]
    canvas = canvas.reshape(HC * WC, ES // 2)
    canvas = np.ascontiguousarray(np.vstack([canvas, np.zeros((1, ES // 2), BF)]))
    # womT: lhsT per tap, quadrant-replicated 27 output rows; tap KK is the
    # rank-3 affine fold (row0: y-base indicator, row1: x-base indicator,
    # row2: per-row constant = tap shift + offset bias)
    wsel = np.zeros((32, CIN, K, K), f32)
    for j in range(9):
        wsel[j] = offset_w[2 * j]
        wsel[9 + j] = offset_w[2 * j + 1]
        wsel[18 + j] = mod_w[j]
    womT = np.zeros((128, KK + 1, 128), f32)
    for t in range(KK):
        blk = wsel[:, :, t // 3, t % 3].T                             # [CIN, 32]
        for q in range(4):
            womT[:, t, 32 * q:32 * q + 32] = blk
    aff = np.zeros((128, 128), f32)
    for q in range(4):
        for j in range(9):
            aff[0, 32 * q + j] = 1.0                                  # y rows: + A
            aff[1, 32 * q + 9 + j] = 1.0                              # x rows: + C
            aff[2, 32 * q + j] = (j // 3) + offset_b[2 * j]
            aff[2, 32 * q + 9 + j] = (j % 3) + offset_b[2 * j + 1]
    womT[:, KK, :] = aff
    womT = womT.astype(BF)
    # wmnT: lhsT per (tap, m-half) in bf16; the mask's 2x is folded in here
    # (device computes sigmoid, not 2*sigmoid)
    wmnT = np.zeros((128, KK * 2, 128), BF)
    for t in range(KK):
        wt = 2.0 * weight[:, :, t // 3, t % 3]                        # [COUT, CIN]
        for m in range(2):
            wmnT[:, t * 2 + m, :] = wt[m * 128:(m + 1) * 128, :].T.astype(BF)
    # mask bias (for the sigmoid), quadrant-replicated
    bmsk = np.zeros((32, 1), f32)
    for j in range(9):
        bmsk[18 + j, 0] = mod_b[j]
    biasmsk = np.tile(bmsk, (4, 1))
    # affine rhs rows: A = r + h*32 + PADC - 1, C = w + PADC - 1, ones
    pp = np.arange(N)
    ar = np.zeros((4, N), f32)
    ar[0] = pp // W + h * HALF + PADC - 1
    ar[1] = pp % W + PADC - 1
    ar[2] = 1.0
    arhs = ar.astype(BF)
    return {
        "xpad": xpad.reshape(128, 34 * 66),
        "canvas": canvas,
        "womT": womT.reshape(128, (KK + 1) * 128),
        "wmnT": wmnT.reshape(128, KK * 2 * 128),
        "biasmsk": biasmsk,
        "arhs": arhs,
    }


def make_in_maps(x, offset_w, offset_b, mod_w, mod_b, weight):
    return [
        _prep_core_inputs(x, offset_w, offset_b, mod_w, mod_b, weight,
                          core // 2, core % 2)
        for core in range(NCORES)
    ]


def get_program(debug=False):
    key = ("nc", debug)
    if key not in _cache:
        _cache[key] = _build_program(debug)
    return _cache[key]


def assemble_output(results):
    out = np.zeros((B, COUT, H, W), np.float32)
    for core in range(NCORES):
        b, h = core // 2, core % 2
        r = np.asarray(results[core]["out"], dtype=np.float32)       # [2,128,N]
        out[b, :, h * HALF:(h + 1) * HALF, :] = r.reshape(COUT, HALF, W)
    return out


def kernel(x, offset_w, offset_b, mod_w, mod_b, weight):
    x = np.asarray(x, np.float32)
    offset_w = np.asarray(offset_w, np.float32)
    offset_b = np.asarray(offset_b, np.float32)
    mod_w = np.asarray(mod_w, np.float32)
    mod_b = np.asarray(mod_b, np.float32)
    weight = np.asarray(weight, np.float32)
    nc = get_program()
    in_maps = make_in_maps(x, offset_w, offset_b, mod_w, mod_b, weight)
    try:
        res = run_bass_kernel_spmd(nc, in_maps, list(range(NCORES)))
    except Exception:
        # transient NRT_EXEC_UNIT_UNRECOVERABLE can occur if the device is
        # mid-reset from a previous process; one retry after a pause recovers
        import time
        time.sleep(20)
        res = run_bass_kernel_spmd(nc, in_maps, list(range(NCORES)))
    return assemble_output(res.results)
